# revision 18
# baseline (speedup 1.0000x reference)
"""BirthDeathIntervalLoss on 8 Trainium2 NeuronCores.

Strategy: the loss only reads 2*B*C*N*2 = 32768 scattered elements of the
512x512 prediction maps. Instead of streaming all 134MB, each core:
  1. DMAs its shard's interval tensors (48KB packed) into 8 SBUF partitions,
  2. computes flat gather indices on the vector engine,
  3. gathers the 4096 needed prediction values with 8 indirect DMAs
     (one 4-byte descriptor per value; offsets for call k are the
     contiguous row idx[k, :], dest is row g[k, :]),
  4. computes sum(W[s,c,n] * (birth-death)^2) + const on-chip,
  5. writes one partial scalar.
The host sums the 8 partials (data-parallel all-reduce of the scalar loss).

The masked-mean algebra of the reference folds into a constant per-(set,
class, n) weight map plus an additive constant:
  loss = sum_{s,b,c,n} W[s,c,n] * (birth-death)^2 + B * sum_s a_s*BETA*cnt_s/C
  W[s,c,n] = a_s * (-BETA/good_s[c] if n < good_s[c] else (1-BETA)/(N-good_s[c])) / C
with a_0 = ALPHA, a_1 = 1-ALPHA, cnt_s = #{c : good_s[c] > 0}.

Value numbering: u = 2*m + e, m = pair id in (set, batch, class, n) order,
e = 0 birth / 1 death. Gather k = u // 512 handles value u. The offset
walker always strides all 128 partitions of the offset AP (free column
advances every 128 entries), so value u's offset sits at
idx[(u%512) % 128, 4k + (u%512)//128]; its gathered value lands at
g[k, u % 512]. Pair m then lives at (g[m//256, 2*(m%256)], ...+1).
"""

import numpy as np

# ---- problem constants (hardcoded per harness contract) ----
B, C, H, W, N = 32, 4, 512, 512, 64
GOOD = np.array([[1, 2, 1, 3], [1, 0, 2, 1]], dtype=np.int64)  # [set, class]
ALPHA = 0.5
BETA = 0.5
N_CORES = 8
B_LOC = B // N_CORES  # 4 batches per core

PRED_LOC = B_LOC * C * H * W          # 4,194,304 f32 per core
N_VALS = 2 * B_LOC * C * N * 2        # 4096 gathered values per core
N_PAIRS = N_VALS // 2                 # 2048 (birth,death) pairs per core

KG = 8                                # independent indirect DMAs
P = 128                               # offset-tile partitions
FV = N_VALS // P                      # 32 offset columns
FI = 2 * FV                           # 64 iv ints per partition
# offset columns per gather call; a small final call shortens the
# completion tail after the last (serial) descriptor-generation slice
_KCOLS = [5, 5, 5, 5, 4, 4, 3, 1]
assert sum(_KCOLS) == FV and len(_KCOLS) == KG
_KSTART = np.cumsum([0] + _KCOLS)     # column offsets per call
QGMAX = max(_KCOLS) * P               # 640: padded row length of g
QP = QGMAX // 2                       # 320 pairs per padded row


def _host_constants():
    """Weight map [KG, QP] f32 (wts[k, j] = weight of pair m = k*QP + j)
    and the per-core additive constant."""
    a = np.array([ALPHA, 1.0 - ALPHA])
    m = np.arange(N_PAIRS)
    s = m // (B_LOC * C * N)
    cc = (m // N) % C
    n = m % N
    g = GOOD[s, cc]
    w = np.where(
        n < g,
        -a[s] * BETA / np.maximum(g, 1) / C,
        a[s] * (1.0 - BETA) / (N - g) / C,
    ).astype(np.float32)
    # pair m -> (call k, slot j); zero-pad rows shorter than QGMAX
    sizes = np.array(_KCOLS) * P // 2           # pairs per call
    starts = np.cumsum(np.concatenate([[0], sizes]))
    wts = np.zeros((KG, QP), dtype=np.float32)
    for k in range(KG):
        wts[k, : sizes[k]] = w[starts[k] : starts[k + 1]]

    cnt = (GOOD > 0).sum(axis=1)  # per set
    const_per_core = float((a * BETA * cnt / C).sum() * B_LOC)
    return wts, const_per_core


_WTS, _CONST = _host_constants()

# value u -> (pair m, endpoint e) -> (set, batch, class, n) natural order.
_U = np.arange(N_VALS)
_M = _U // 2
_BB = (_M // (C * N)) % B_LOC
_CC = (_M // N) % C
_IMGBASE = ((_BB * C + _CC) * (H * W)).astype(np.int32)  # per value

# offset-tile position of value u: call k = searchsorted over cumulative
# sizes; within-call index u' -> (u' % P, kstart + u' // P)
_VSIZES = np.array(_KCOLS) * P               # values per call
_VSTARTS = np.cumsum(np.concatenate([[0], _VSIZES]))
_KOF = np.searchsorted(_VSTARTS, _U, side="right") - 1  # call of value u
_UP = _U - _VSTARTS[_KOF]                               # index within call
_IPOS_P = _UP % P
_IPOS_F = _KSTART[_KOF] + _UP // P
_BASE = np.zeros((P, FV), dtype=np.int32)
_BASE[_IPOS_P, _IPOS_F] = _IMGBASE
_IV_POS_R = _IPOS_P * FI + 2 * _IPOS_F

_PROGRAM = None
_LAST_RESULTS = None  # BassKernelResults of the most recent run (for test.py)
TRACE = False


def _build_program():
    from concourse import bacc, mybir
    import concourse.bass as bass
    import concourse.tile as tile

    f32 = mybir.dt.float32
    i32 = mybir.dt.int32

    nc = bacc.Bacc("TRN2", target_bir_lowering=False, debug=False)

    pred_d = nc.dram_tensor("pred", [PRED_LOC], f32, kind="ExternalInput")
    ivb_d = nc.dram_tensor("ivb", [P, FI + FV], i32, kind="ExternalInput")
    wts_d = nc.dram_tensor("wts", [KG, QP], f32, kind="ExternalInput")
    out_d = nc.dram_tensor("out", [1, 1], f32, kind="ExternalOutput")

    with tile.TileContext(nc) as tc:
        with (
            tc.tile_pool(name="sb", bufs=1) as pool,
            tc.tile_pool(name="ps", bufs=1, space="PSUM") as psp,
        ):
            # packed iv rows (cols 0:2Q) + image base (cols 2Q:3Q), one DMA;
            # weights ride the scalar engine's HWDGE ring in parallel.
            ivb = pool.tile([P, FI + FV], i32)
            nc.sync.dma_start(ivb[:], ivb_d[:])
            wts = pool.tile([KG, QP], f32)
            nc.scalar.dma_start(wts[:], wts_d[:])

            # idx = row * W + image_base + col   (offset-tile positions)
            idx = pool.tile([P, FV], i32)
            nc.vector.tensor_scalar(
                out=idx[:],
                in0=ivb[:, 0:FI:2],
                scalar1=9,  # W == 512 == 1 << 9
                scalar2=None,
                op0=mybir.AluOpType.logical_shift_left,
            )
            nc.vector.tensor_tensor(
                out=idx[:], in0=idx[:], in1=ivb[:, 1:FI:2],
                op=mybir.AluOpType.add,
            )
            nc.vector.tensor_tensor(
                out=idx[:], in0=idx[:], in1=ivb[:, FI : FI + FV],
                op=mybir.AluOpType.add,
            )

            # KG indirect DMAs; call k consumes its offset columns
            # partition-fastest and writes g[k, :n_k] (one descriptor per
            # value). Rows shorter than QGMAX stay garbage; their weights
            # are zero.
            g = pool.tile([KG, QGMAX], f32)
            src = pred_d.ap().rearrange("(a f) -> a f", a=1)
            for k in range(KG):
                c0, c1 = int(_KSTART[k]), int(_KSTART[k + 1])
                nvals = (c1 - c0) * P
                nc.gpsimd.indirect_dma_start(
                    out=g[k : k + 1, 0:nvals].rearrange(
                        "a (f one) -> a f one", one=1
                    ),
                    out_offset=None,
                    in_=src,
                    in_offset=bass.IndirectOffsetOnAxis(
                        ap=idx[:, c0:c1], axis=1
                    ),
                )

            # pair m at (g[k, 2j], g[k, 2j+1]), weight wts[k, j]
            d = pool.tile([KG, QP], f32)
            nc.vector.tensor_tensor(
                out=d[:], in0=g[:, 0:QGMAX:2], in1=g[:, 1:QGMAX:2],
                op=mybir.AluOpType.subtract,
            )
            dw = pool.tile([KG, QP], f32)
            nc.vector.tensor_tensor(
                out=dw[:], in0=d[:], in1=d[:], op=mybir.AluOpType.mult
            )
            dw2 = pool.tile([KG, QP], f32)
            nc.vector.tensor_tensor(
                out=dw2[:], in0=dw[:], in1=wts[:], op=mybir.AluOpType.mult
            )
            r = pool.tile([KG, 1], f32)
            nc.vector.reduce_sum(out=r[:], in_=dw2[:], axis=mybir.AxisListType.X)

            ones = pool.tile([KG, 1], f32)
            nc.vector.memset(ones[:], 1.0)
            acc = psp.tile([1, 1], f32)
            nc.tensor.matmul(acc[:], lhsT=r[:], rhs=ones[:], start=True, stop=True)
            cst = pool.tile([1, 1], f32)
            nc.vector.memset(cst[:], _CONST)
            res = pool.tile([1, 1], f32)
            nc.vector.tensor_tensor(
                out=res[:], in0=acc[:], in1=cst[:], op=mybir.AluOpType.add
            )
            nc.sync.dma_start(out_d[:], res[:])

    nc.compile()
    return nc


def _get_program():
    global _PROGRAM
    if _PROGRAM is None:
        _PROGRAM = _build_program()
    return _PROGRAM


def kernel(prediction, intervals_comp_0, intervals_comp_1):
    global _LAST_RESULTS
    from concourse.bass_utils import run_bass_kernel_spmd

    nc = _get_program()

    prediction = np.asarray(prediction, dtype=np.float32)
    i0 = np.asarray(intervals_comp_0, dtype=np.int32)
    i1 = np.asarray(intervals_comp_1, dtype=np.int32)

    in_maps = []
    for mcore in range(N_CORES):
        sl = slice(mcore * B_LOC, (mcore + 1) * B_LOC)
        iv_nat = np.stack([i0[sl], i1[sl]])  # [2, B_LOC, C, N, 2, 2]
        rc = iv_nat.reshape(N_VALS, 2)       # value u -> (r, c)
        iv_flat = np.empty(P * FI, dtype=np.int32)
        iv_flat[_IV_POS_R] = rc[:, 0]
        iv_flat[_IV_POS_R + 1] = rc[:, 1]
        ivb = np.empty((P, FI + FV), dtype=np.int32)
        ivb[:, :FI] = iv_flat.reshape(P, FI)
        ivb[:, FI:] = _BASE
        in_maps.append(
            {
                "pred": np.ascontiguousarray(prediction[sl]).reshape(-1),
                "ivb": ivb,
                "wts": _WTS,
            }
        )

    results = run_bass_kernel_spmd(
        nc, in_maps, list(range(N_CORES)), trace=TRACE
    )
    _LAST_RESULTS = results
    total = sum(float(r["out"][0, 0]) for r in results.results)
    return np.array(total, dtype=np.float32)
